# revision 1
# baseline (speedup 1.0000x reference)
"""GQA attention (B=2,T=2048,D=2048,H=16,KV=4,HD=128, causal+RoPE) on 8 trn2 cores.

Sharding: 4-way head tensor-parallel x 2-way batch data-parallel.
Core c: batch b=c//4, TP shard s=c%4 -> q heads [4s..4s+3], kv head s.
Host pre-transposes x -> xT (bf16), folds the RoPE even/odd gather into a
column permutation of Wq/Wk, folds the 1/sqrt(HD) score scale into the q
RoPE tables, and sums the 4 TP partial outputs per batch at the end.

Device per core (all matmuls bf16 with fp32 PSUM accumulation):
  qT/kT = W^T @ x^T via PE (heads on partitions), RoPE on DVE,
  V = x @ Wv via PE (tokens on partitions),
  causal flash attention without max-subtraction (scores are small),
  P^T via PE transpose, PV accumulation, per-partition 1/l normalize,
  O^T via PE transpose, Wo matmul -> fp32 output tiles -> DRAM.
"""

import math
import os
import numpy as np

try:
    import concourse.bass as bass
except ImportError:  # pragma: no cover
    import sys

    sys.path.insert(0, "/opt/trn_rl_repo")
    import concourse.bass as bass

import concourse.mybir as mybir
import concourse.bacc as bacc
from concourse import bass_utils
from concourse.tile import TileContext
from contextlib import ExitStack
from ml_dtypes import bfloat16

B, T, D = 2, 2048, 2048
H, KV, HD = 16, 4, 128
TP = 4  # head-TP ways
NH = H // TP  # q heads per core = 4
NKB = D // 128  # 16 contraction blocks
NTC = T // 512  # 4 free-dim chunks
NTB = T // 128  # 16 token blocks
SCALE = 1.0 / math.sqrt(HD)
F32 = mybir.dt.float32
BF16 = mybir.dt.bfloat16
EXP = mybir.ActivationFunctionType.Exp
MASK_VAL = -1e9

_program = None
_last_results = None
last_exec_time_ns = None


def _build_program():
    global _program
    if _program is not None:
        return _program

    nc = bacc.Bacc(
        "TRN2",
        target_bir_lowering=False,
        debug=False,
        enable_asserts=False,
        num_devices=8,
    )
    xT_d = nc.dram_tensor("xT", [D, T], BF16, kind="ExternalInput").ap()
    wq_d = nc.dram_tensor("Wq", [D, NH * HD], BF16, kind="ExternalInput").ap()
    wk_d = nc.dram_tensor("Wk", [D, HD], BF16, kind="ExternalInput").ap()
    wv_d = nc.dram_tensor("Wv", [D, HD], BF16, kind="ExternalInput").ap()
    wo_d = nc.dram_tensor("Wo", [NH * HD, D], BF16, kind="ExternalInput").ap()
    cq_d = nc.dram_tensor("cosq", [128, T], F32, kind="ExternalInput").ap()
    sq_d = nc.dram_tensor("sinq", [128, T], F32, kind="ExternalInput").ap()
    ck_d = nc.dram_tensor("cosk", [128, T], F32, kind="ExternalInput").ap()
    sk_d = nc.dram_tensor("sink", [128, T], F32, kind="ExternalInput").ap()
    mask_d = nc.dram_tensor("mask", [128, 128], F32, kind="ExternalInput").ap()
    id_d = nc.dram_tensor("ident", [128, 128], BF16, kind="ExternalInput").ap()
    y_d = nc.dram_tensor("y", [T, D], BF16, kind="ExternalOutput").ap()

    with TileContext(nc) as tc, ExitStack() as ctx:
        big = ctx.enter_context(tc.tile_pool(name="big", bufs=1))
        ps = ctx.enter_context(tc.tile_pool(name="ps", bufs=4, space="PSUM"))
        ps_t = ctx.enter_context(tc.tile_pool(name="ps_t", bufs=2, space="PSUM"))
        ps_o = ctx.enter_context(tc.tile_pool(name="ps_o", bufs=2, space="PSUM"))
        rtmp = ctx.enter_context(tc.tile_pool(name="rtmp", bufs=3))
        ppool = ctx.enter_context(tc.tile_pool(name="ppool", bufs=2))
        ptpool = ctx.enter_context(tc.tile_pool(name="ptpool", bufs=16))
        opool = ctx.enter_context(tc.tile_pool(name="opool", bufs=2))
        apool = ctx.enter_context(tc.tile_pool(name="apool", bufs=4))
        ypool = ctx.enter_context(tc.tile_pool(name="ypool", bufs=3))

        xT = big.tile([128, NKB, T], BF16, tag="xT")
        wq = big.tile([128, NKB, NH * HD], BF16, tag="wq")
        wk = big.tile([128, NKB, HD], BF16, tag="wk")
        wv = big.tile([128, NKB, HD], BF16, tag="wv")
        wo = big.tile([128, NH, D], BF16, tag="wo")
        cq = big.tile([128, T], F32, tag="cq")
        sq = big.tile([128, T], F32, tag="sq")
        ck = big.tile([128, T], F32, tag="ck")
        sk = big.tile([128, T], F32, tag="sk")
        mask = big.tile([128, 128], F32, tag="mask")
        ident = big.tile([128, 128], BF16, tag="ident")
        qT = big.tile([128, NH, T], BF16, tag="qT")
        kT = big.tile([128, T], BF16, tag="kT")
        V = big.tile([128, NTB, HD], BF16, tag="V")
        OT = big.tile([128, NH, T], BF16, tag="OT")

        # ---- loads ----
        for j in range(NKB):
            nc.sync.dma_start(out=xT[:, j, :], in_=xT_d[j * 128 : (j + 1) * 128, :])
            nc.sync.dma_start(out=wq[:, j, :], in_=wq_d[j * 128 : (j + 1) * 128, :])
            nc.sync.dma_start(out=wk[:, j, :], in_=wk_d[j * 128 : (j + 1) * 128, :])
            nc.sync.dma_start(out=wv[:, j, :], in_=wv_d[j * 128 : (j + 1) * 128, :])
        for h in range(NH):
            nc.sync.dma_start(out=wo[:, h, :], in_=wo_d[h * 128 : (h + 1) * 128, :])
        for d_ap, s_tile in ((cq_d, cq), (sq_d, sq), (ck_d, ck), (sk_d, sk)):
            nc.sync.dma_start(out=s_tile[:, :1024], in_=d_ap[:, :1024])
            nc.sync.dma_start(out=s_tile[:, 1024:], in_=d_ap[:, 1024:])
        nc.sync.dma_start(out=mask[:], in_=mask_d[:])
        nc.sync.dma_start(out=ident[:], in_=id_d[:])

        # ---- projections with fused RoPE (heads on partitions) ----
        def proj_rope(w_sb, w_col0, cos_sb, sin_sb, dst):
            for tcc in range(NTC):
                sl = slice(tcc * 512, (tcc + 1) * 512)
                pst = ps.tile([128, 512], F32, tag="ps")
                for j in range(NKB):
                    nc.tensor.matmul(
                        pst[:],
                        lhsT=w_sb[:, j, w_col0 : w_col0 + 128],
                        rhs=xT[:, j, sl],
                        start=(j == 0),
                        stop=(j == NKB - 1),
                    )
                t1 = rtmp.tile([128, 512], F32, tag="t1")
                nc.vector.tensor_mul(t1[:], pst[:], cos_sb[:, sl])
                t2 = rtmp.tile([128, 512], F32, tag="t2")
                nc.vector.tensor_mul(t2[0:64, :], pst[64:128, :], sin_sb[0:64, sl])
                nc.vector.tensor_mul(t2[64:128, :], pst[0:64, :], sin_sb[64:128, sl])
                nc.vector.tensor_add(dst[:, sl], t1[:], t2[:])

        proj_rope(wk, 0, ck, sk, kT)
        for h in range(NH):
            proj_rope(wq, h * 128, cq, sq, qT[:, h, :])

        # ---- V projection (tokens on partitions) ----
        for tb in range(NTB):
            pst = ps.tile([128, 128], F32, tag="ps")
            for j in range(NKB):
                nc.tensor.matmul(
                    pst[:],
                    lhsT=xT[:, j, tb * 128 : (tb + 1) * 128],
                    rhs=wv[:, j, :],
                    start=(j == 0),
                    stop=(j == NKB - 1),
                )
            nc.scalar.copy(V[:, tb, :], pst[:])

        # ---- causal attention + output projection ----
        for i in range(NTB):
            L = (i + 1) * 128
            ncn = (L + 511) // 512
            for h in range(NH):
                P = ppool.tile([128, T], BF16, tag="P")
                acc = apool.tile([128, 4], F32, tag="acc")
                for c in range(ncn):
                    Nc = min(512, L - c * 512)
                    spt = ps.tile([128, 512], F32, tag="ps")
                    nc.tensor.matmul(
                        spt[:, :Nc],
                        lhsT=qT[:, h, i * 128 : (i + 1) * 128],
                        rhs=kT[:, c * 512 : c * 512 + Nc],
                        start=True,
                        stop=True,
                    )
                    if c == ncn - 1:
                        nc.vector.tensor_add(
                            spt[:, Nc - 128 : Nc], spt[:, Nc - 128 : Nc], mask[:]
                        )
                    nc.scalar.activation(
                        P[:, c * 512 : c * 512 + Nc],
                        spt[:, :Nc],
                        EXP,
                        accum_out=acc[:, c : c + 1],
                    )
                l_t = apool.tile([128, 1], F32, tag="l")
                nc.vector.reduce_sum(l_t[:], acc[:, :ncn], axis=mybir.AxisListType.X)
                rl = apool.tile([128, 1], F32, tag="rl")
                nc.vector.reciprocal(rl[:], l_t[:])

                pts = []
                for b in range(i + 1):
                    ptp = ps_t.tile([128, 128], BF16, tag="ps_t")
                    nc.tensor.transpose(ptp[:], P[:, b * 128 : (b + 1) * 128], ident[:])
                    pt_sb = ptpool.tile([128, 128], BF16, tag="pt")
                    nc.vector.tensor_copy(pt_sb[:], ptp[:])
                    pts.append(pt_sb)
                opst = ps_o.tile([128, 128], F32, tag="ps_o")
                for b in range(i + 1):
                    nc.tensor.matmul(
                        opst[:],
                        lhsT=pts[b][:],
                        rhs=V[:, b, :],
                        start=(b == 0),
                        stop=(b == i),
                    )
                osb = opool.tile([128, 128], BF16, tag="o")
                nc.vector.tensor_scalar_mul(osb[:], opst[:], rl[:])
                otp = ps_t.tile([128, 128], BF16, tag="ps_t")
                nc.tensor.transpose(otp[:], osb[:], ident[:])
                nc.vector.tensor_copy(OT[:, h, i * 128 : (i + 1) * 128], otp[:])

            for dc in range(4):
                wpst = ps.tile([128, 512], F32, tag="ps")
                for h in range(NH):
                    nc.tensor.matmul(
                        wpst[:],
                        lhsT=OT[:, h, i * 128 : (i + 1) * 128],
                        rhs=wo[:, h, dc * 512 : (dc + 1) * 512],
                        start=(h == 0),
                        stop=(h == NH - 1),
                    )
                ysb = ypool.tile([128, 512], BF16, tag="y")
                nc.scalar.copy(ysb[:], wpst[:])
                nc.sync.dma_start(
                    out=y_d[i * 128 : (i + 1) * 128, dc * 512 : (dc + 1) * 512],
                    in_=ysb[:],
                )

    nc.compile()
    _program = nc
    return nc


def _host_prep(x, Wq, Wk, Wv, Wo):
    x = np.asarray(x, dtype=np.float32)
    Wq = np.asarray(Wq, dtype=np.float32)
    Wk = np.asarray(Wk, dtype=np.float32)
    Wv = np.asarray(Wv, dtype=np.float32)
    Wo = np.asarray(Wo, dtype=np.float32)

    # RoPE even/odd gather folded into weight column permutation (per head)
    perm128 = np.r_[np.arange(0, 128, 2), np.arange(1, 128, 2)]
    permq = np.concatenate([hb * 128 + perm128 for hb in range(H)])
    permk = np.concatenate([hb * 128 + perm128 for hb in range(KV)])
    Wq_p = Wq[:, permq]
    Wk_p = Wk[:, permk]

    pos = np.arange(T, dtype=np.float64)
    inv_freq = 1.0 / (10000.0 ** (np.arange(0, HD, 2, dtype=np.float64) / HD))
    ang = np.einsum("t,f->tf", pos, inv_freq)  # [T, 64]
    cos = np.cos(ang).T.astype(np.float32)  # [64, T]
    sin = np.sin(ang).T.astype(np.float32)
    cosk = np.concatenate([cos, cos], axis=0)  # [128, T]
    sink = np.concatenate([-sin, sin], axis=0)
    cosq = (cosk * SCALE).astype(np.float32)
    sinq = (sink * SCALE).astype(np.float32)

    mask = np.triu(np.full((128, 128), MASK_VAL, dtype=np.float32), k=1)
    ident = np.eye(128, dtype=bfloat16)

    in_maps = []
    for c in range(8):
        b, s = c // 4, c % 4
        in_maps.append(
            {
                "xT": np.ascontiguousarray(x[b].T).astype(bfloat16),
                "Wq": Wq_p[:, s * 512 : (s + 1) * 512].astype(bfloat16),
                "Wk": Wk_p[:, s * 128 : (s + 1) * 128].astype(bfloat16),
                "Wv": Wv[:, s * 128 : (s + 1) * 128].astype(bfloat16),
                "Wo": np.ascontiguousarray(Wo[s * 512 : (s + 1) * 512, :]).astype(
                    bfloat16
                ),
                "cosq": cosq,
                "sinq": sinq,
                "cosk": cosk,
                "sink": sink,
                "mask": mask,
                "ident": ident,
            }
        )
    return in_maps


def _ensure_ntff_hook():
    """The agent image's antenv lacks axon_hooks, so boot() skips installing
    the NTFF profile hook. Recreate the module and install the hook."""
    import sys
    import types

    try:
        from antenv.axon_hooks import get_axon_ntff_profile_hook  # noqa: F401

        return True
    except ImportError:
        pass
    try:
        import antenv
        from trn_agent_boot.trn_boot import _ntff_profile_via_ctypes

        hook = _ntff_profile_via_ctypes("/opt/axon/libaxon_pjrt.so")
        if hook is None:
            return False
        mod = types.ModuleType("antenv.axon_hooks")
        mod._hook = hook
        mod.set_axon_ntff_profile_hook = lambda h: setattr(mod, "_hook", h)
        mod.get_axon_ntff_profile_hook = lambda: mod._hook
        sys.modules["antenv.axon_hooks"] = mod
        antenv.axon_hooks = mod
        bass_utils.upload_artifacts = lambda d: d
        return True
    except Exception:
        return False


def kernel(x, Wq, Wk, Wv, Wo):
    global _last_results, last_exec_time_ns
    nc = _build_program()
    in_maps = _host_prep(x, Wq, Wk, Wv, Wo)
    trace = bool(int(os.environ.get("KERNEL_TRACE", "0")))
    tmpdir = None
    if trace:
        trace = _ensure_ntff_hook()
        if trace:
            tmpdir = os.environ.get("KERNEL_TRACE_DIR") or None
    res = bass_utils.run_bass_kernel_spmd(
        nc, in_maps, core_ids=list(range(8)), trace=trace, tmpdir=tmpdir
    )
    _last_results = res
    last_exec_time_ns = res.exec_time_ns
    out = np.empty((B, T, D), dtype=np.float32)
    for b in range(B):
        out[b] = sum(
            res.results[4 * b + s]["y"].astype(np.float32) for s in range(TP)
        )
    return out

